# revision 14
# baseline (speedup 1.0000x reference)
"""Multi-head self-attention (RoPE, causal) TRN2 Bass kernel — v2.

Problem: B=4, S=2048, D=1024, H=16, Dh=64, fp32 in/out.

Sharding (8 cores): DP=4 over batch x TP=2 over heads (Megatron-style).
Core c handles batch c//2 with heads (c%2)*8 .. (c%2)*8+7 and produces a
partial output [S, D] (stored transposed, bf16); the host sums the two TP
partials per batch (the all-reduce after out_projection).

v2 changes vs v1 (386.7us):
  * All SBUF operands bf16: enables PE Fast-Weight-Load (4x LDWEIGHTS),
    2x DVE elementwise, half DMA / SBUF traffic. PSUM accumulation and
    softmax denominators stay fp32. (rel-err budget is ~67x measured v1.)
  * Globally software-pipelined emission: QKV(ts+1), attention row ts,
    norm(ts-1) and out-projection are interleaved chunk-wise so the
    in-order PE queue always has independent matmuls while ACT (exp) is
    the per-row bottleneck, and HAM never re-throttles.
  * PE + ACT warmup at t=0 (HAM un-throttle + exp table preload) under
    the initial DMA fill; W DMA split column-part-first so the first
    QKV chain unlocks after ~2MB instead of 8MB.
  * Scalar engine runs ONLY exp; copies pinned to DVE, RoPE swaps +
    causal masks + denominator gathers on GpSimd.
  * Norm: per-pair broadcast (one [8x128] one-hot matmul per pair) after
    repacking odd-head ctx, one in-place [128,512] bf16 multiply.
"""

import sys

for _p in ("/opt/trn_rl_repo", "/root/.axon_site/_ro/trn_rl_repo"):
    if _p not in sys.path:
        sys.path.insert(0, _p)

import numpy as np
import ml_dtypes

import concourse.bacc as bacc
import concourse.bass_utils as bass_utils
import concourse.mybir as mybir
import concourse.tile as tile
from concourse.bass_utils import run_bass_kernel_spmd

F32 = mybir.dt.float32
F32R = mybir.dt.float32r
BF16 = mybir.dt.bfloat16
EXP = mybir.ActivationFunctionType.Exp
BF = ml_dtypes.bfloat16

B, S, D = 4, 2048, 1024
H, DH = 16, 64
THETA = 10000.0
NCORES, TP, HLOC = 8, 2, 8          # 8 local heads per core, 4 pairs
NPAIR = HLOC // 2
NT = S // 512                        # 4 q/t tiles of 512
NTQ = S // 128                       # 16 t-chunks of 128
ND = D // 128                        # 8 d-chunks
SCALE = 1.0 / 8.0                    # 1/sqrt(DH)
VW = DH + 2                          # v row stride (64 dv + ones + pad)

_PROGRAM = None


def _merge_emit(*streams):
    """Interleave chunk streams proportionally by estimated ns.

    Each stream is a list of (est_ns, closure). Emission order within a
    stream is preserved; across streams we pace by fraction-completed so
    the instruction queues see a balanced mix.
    """
    streams = [list(s) for s in streams if s]
    totals = [max(1.0, sum(c for c, _ in s)) for s in streams]
    done = [0.0] * len(streams)
    idx = [0] * len(streams)
    while True:
        best, bestf = -1, None
        for k, s in enumerate(streams):
            if idx[k] >= len(s):
                continue
            f = done[k] / totals[k]
            if bestf is None or f < bestf:
                best, bestf = k, f
        if best < 0:
            return
        cost, fn = streams[best][idx[best]]
        fn()
        done[best] += cost
        idx[best] += 1


def _build_program():
    nc = bacc.Bacc(None)

    xT_d = nc.dram_tensor("xT", [D, S], BF16, kind="ExternalInput")
    wqkvT_d = nc.dram_tensor("wqkvT", [D, 3 * HLOC * DH], BF16, kind="ExternalInput")
    woT_d = nc.dram_tensor("woT", [NPAIR, 128, D], BF16, kind="ExternalInput")
    cos_d = nc.dram_tensor("cosT", [128, S], BF16, kind="ExternalInput")
    sin_d = nc.dram_tensor("sinT", [128, S], BF16, kind="ExternalInput")
    mask_d = nc.dram_tensor("mask", [128, 2 * 128], BF16, kind="ExternalInput")
    oh2_d = nc.dram_tensor("oh2", [8, 4 * 128], F32R, kind="ExternalInput")
    out_d = nc.dram_tensor("out", [D, S], BF16, kind="ExternalOutput")

    with tile.TileContext(nc) as tc:
        with (
            tc.tile_pool(name="const", bufs=1) as constp,
            tc.tile_pool(name="qkpool", bufs=1) as qkpool,
            tc.tile_pool(name="vpool", bufs=1) as vpool,
            tc.tile_pool(name="wpool", bufs=1) as wpool,
            tc.tile_pool(name="xpool", bufs=1) as xpool,
            tc.tile_pool(name="ropep", bufs=1) as ropep,
            tc.tile_pool(name="ctxbp", bufs=1) as ctxbp,
            tc.tile_pool(name="ptpool", bufs=1) as ptpool,
            tc.tile_pool(name="nrmpool", bufs=1) as nrmpool,
            tc.tile_pool(name="otpool", bufs=1) as otpool,
            tc.tile_pool(name="wopool", bufs=1) as wopool,
            tc.tile_pool(name="ps_sm", bufs=1, space="PSUM") as ps_sm,
            tc.tile_pool(name="ps_st", bufs=1, space="PSUM") as ps_st,
            tc.tile_pool(name="ps_ctx", bufs=1, space="PSUM") as ps_ctx,
        ):
            # ---------------- warmup: HAM un-throttle + ACT table ----------
            wmov = constp.tile([128, 512], BF16)
            nc.vector.memset(wmov[:], 0.001)
            warm_ps = ps_sm.tile([128, 512], F32, tag="sm", bufs=2, name="warm")
            for k in range(10):
                nc.tensor.matmul(
                    warm_ps[:], wmov[:, (k % 4) * 128:(k % 4 + 1) * 128],
                    wmov[:], start=True, stop=True)
            dumex = constp.tile([128, 32], BF16)
            nc.scalar.activation(dumex[:], wmov[:, 0:32], EXP, scale=0.125)

            # ---------------- persistent tiles ----------------
            qt = [qkpool.tile([128, S], BF16, name=f"qt{p}") for p in range(NPAIR)]
            kt = [qkpool.tile([128, S], BF16, name=f"kt{p}") for p in range(NPAIR)]
            vt = [vpool.tile([128, HLOC, VW], BF16, name=f"v{t}") for t in range(NTQ)]
            ctxb = [ctxbp.tile([65, S], BF16, name=f"ctxb{p}") for p in range(NPAIR)]
            w_sb = [wpool.tile([128, 3 * HLOC * DH], BF16, name=f"w{d}") for d in range(ND)]
            cos_sb = constp.tile([128, S], BF16, name="cos")
            sin_sb = constp.tile([128, S], BF16, name="sin")
            mask_sb = constp.tile([128, 2, 128], BF16, name="mask")
            oh2_sb = constp.tile([8, 4 * 128], F32R, name="oh2")
            wo_sb = [wopool.tile([128, D], BF16, name=f"wo{p}") for p in range(NPAIR)]
            den_g = [nrmpool.tile([8, 512], F32, name=f"den{i}") for i in range(NT)]

            xTr = None  # xa tiles come from xpool with tag rotation

            def xa_dma(ts, xa):
                tsl = slice(ts * 512, (ts + 1) * 512)
                for d in range(ND):
                    nc.sync.dma_start(xa[:, d, :], xT_d[d * 128:(d + 1) * 128, tsl])

            # ---------------- initial DMA fill (emission order matters) ----
            xa0 = xpool.tile([128, ND, 512], BF16, tag="x", bufs=2, name="xa0")
            # interleave x(ts=0) with W column-parts so the e=0 chain
            # unlocks after ~2MB of traffic
            for d in range(ND):
                nc.sync.dma_start(xa0[:, d, :], xT_d[d * 128:(d + 1) * 128, 0:512])
                nc.sync.dma_start(w_sb[d][:, 0:512], wqkvT_d[d * 128:(d + 1) * 128, 0:512])
            nc.sync.dma_start(cos_sb[:], cos_d[:])
            nc.sync.dma_start(sin_sb[:], sin_d[:])
            for d in range(ND):
                nc.sync.dma_start(w_sb[d][:, 512:1536],
                                  wqkvT_d[d * 128:(d + 1) * 128, 512:1536])
            nc.scalar.dma_start(mask_sb.rearrange("p h m -> p (h m)"), mask_d[:])
            nc.scalar.dma_start(oh2_sb[:], oh2_d[:])

            # ---------------- chunk generators ----------------
            def qkv_chunks(ts, xa):
                """QKV projection + RoPE for q/t tile ts. ~12 chunks."""
                tsl = slice(ts * 512, (ts + 1) * 512)
                # ScalarE handles the PSUM->SBUF copies while ACT is idle
                # (rounds 0-1); DVE stays free for RoPE / stash work.
                copy_eng = nc.scalar.copy if ts in (1, 2) else nc.vector.tensor_copy
                chunks = []

                def qk_chunk(e):
                    def fn(e=e):
                        ps = ps_sm.tile([128, 512], F32, tag="sm", bufs=2)
                        for d in range(ND):
                            nc.tensor.matmul(
                                ps[:], w_sb[d][:, e * 128:(e + 1) * 128],
                                xa[:, d, :],
                                start=(d == 0), stop=(d == ND - 1),
                            )
                        dst = qt[e] if e < NPAIR else kt[e - NPAIR]
                        copy_eng(dst[:, tsl], ps[:])
                        sw = ropep.tile([128, 512], BF16, tag="sw", bufs=3)
                        for qd in range(4):
                            sq = qd ^ 1
                            nc.gpsimd.dma_start(
                                sw[qd * 32:(qd + 1) * 32, :],
                                dst[sq * 32:(sq + 1) * 32, tsl],
                            )
                        t1 = ropep.tile([128, 512], BF16, tag="t1", bufs=3)
                        nc.vector.tensor_mul(t1[:], dst[:, tsl], cos_sb[:, tsl])
                        nc.vector.tensor_mul(sw[:], sw[:], sin_sb[:, tsl])
                        nc.vector.tensor_add(dst[:, tsl], t1[:], sw[:])
                    return fn

                for e in range(2 * NPAIR):
                    chunks.append((2600.0, qk_chunk(e)))

                def v_chunk(tq0):
                    def fn(tq0=tq0):
                        tq = ts * 4 + tq0
                        psv = ps_sm.tile([128, 512], F32, tag="sm", bufs=2)
                        for d in range(ND):
                            nc.tensor.matmul(
                                psv[:],
                                xa[:, d, tq0 * 128:(tq0 + 1) * 128],
                                w_sb[d][:, 2 * HLOC * DH:3 * HLOC * DH],
                                start=(d == 0), stop=(d == ND - 1),
                            )
                        v = vt[tq]
                        copy_eng(
                            v[:, :, 0:DH],
                            psv.rearrange("p (h d) -> p h d", h=HLOC),
                        )
                        nc.gpsimd.memset(v[:, :, DH:DH + 1], 1.0)
                    return fn

                for tq0 in range(4):
                    chunks.append((2200.0, v_chunk(tq0)))
                return chunks

            def att_chunks(i):
                """Attention row i: 4 pair-units, chunked per kv block j."""
                chunks = []
                nj = 4 * i + 4
                isl = slice(512 * i, 512 * (i + 1))

                for p in range(NPAIR):
                    ctxA = ps_ctx.tile([65, 512], F32, tag="ctx", bufs=2,
                                       name=f"cA{i}_{p}")
                    ctxB = ps_ctx.tile([65, 512], F32, tag="ctx", bufs=2,
                                       name=f"cB{i}_{p}")

                    def j_chunk(p, j, ctxA, ctxB):
                        def fn():
                            lo = max(0, 128 * j - 512 * i)
                            qsl = slice(512 * i + lo, 512 * (i + 1))
                            ksl = slice(j * 128, (j + 1) * 128)
                            st = ps_st.tile([128, 2, 512], F32, tag="st", bufs=2)
                            nc.tensor.matmul(
                                st[:, 0, lo:512], kt[p][0:64, ksl],
                                qt[p][0:64, qsl], tile_position=(0, 0),
                            )
                            nc.tensor.matmul(
                                st[:, 1, lo:512], kt[p][64:128, ksl],
                                qt[p][64:128, qsl], tile_position=(64, 0),
                            )
                            pt = ptpool.tile([128, 2, 512], BF16, tag="pt", bufs=8)
                            nc.scalar.activation(
                                pt[:, :, lo:512], st[:, :, lo:512], EXP,
                                scale=SCALE,
                            )
                            if lo == 128 * j - 512 * i:  # diagonal block
                                nc.gpsimd.tensor_mul(
                                    pt[:, :, lo:lo + 128],
                                    pt[:, :, lo:lo + 128],
                                    mask_sb[:],
                                )
                            nc.tensor.matmul(
                                ctxA[:, lo:512], vt[j][:, 2 * p, 0:DH + 1],
                                pt[:, 0, lo:512],
                                start=(j == 0), stop=(j == nj - 1),
                            )
                            nc.tensor.matmul(
                                ctxB[:, lo:512], vt[j][:, 2 * p + 1, 0:DH + 1],
                                pt[:, 1, lo:512],
                                start=(j == 0), stop=(j == nj - 1),
                            )
                        return fn

                    for j in range(nj):
                        lo = max(0, 128 * j - 512 * i)
                        chunks.append((3.0 * (512 - lo) * 0.42 + 250.0,
                                       j_chunk(p, j, ctxA, ctxB)))

                    def stash(p=p, ctxA=ctxA, ctxB=ctxB):
                        nc.vector.tensor_copy(qt[p][0:65, isl], ctxA[:])
                        nc.vector.tensor_copy(ctxb[p][:, isl], ctxB[:])
                        # fp32 denominators for this pair (bf16 -> f32 cast
                        # DMA is gpsimd-only)
                        nc.gpsimd.dma_start(
                            den_g[i][2 * p:2 * p + 1, :], qt[p][64:65, isl])
                        nc.gpsimd.dma_start(
                            den_g[i][2 * p + 1:2 * p + 2, :], ctxb[p][64:65, isl])
                    chunks.append((700.0, stash))
                return chunks

            def norm_chunks(i):
                """Softmax normalization for row i (runs during row i+1)."""
                isl = slice(512 * i, 512 * (i + 1))
                chunks = []
                rec_f = nrmpool.tile([8, 512], F32, tag="recf", bufs=2,
                                     name=f"recf{i}")
                rec = nrmpool.tile([8, 512], F32R, tag="rec", bufs=2,
                                   name=f"rec{i}")

                def recip():
                    nc.vector.reciprocal_approx_fast(rec_f[:], den_g[i][:])
                    nc.vector.tensor_copy(rec[:], rec_f[:])
                chunks.append((700.0, recip))

                def pair_norm(p):
                    def fn(p=p):
                        # repack odd-head ctx into dead Q_B rows first
                        nc.sync.dma_start(qt[p][64:128, isl], ctxb[p][0:64, isl])
                        bc = ps_sm.tile([128, 512], F32, tag="sm", bufs=2,
                                        name=f"bc{i}_{p}")
                        nc.tensor.matmul(
                            bc[:], oh2_sb[:, p * 128:(p + 1) * 128], rec[:])
                        bc_sb = nrmpool.tile([128, 512], BF16, tag="bcsb", bufs=2)
                        nc.vector.tensor_copy(bc_sb[:], bc[:])
                        nc.vector.tensor_mul(
                            qt[p][:, isl], qt[p][:, isl], bc_sb[:])
                    return fn

                for p in range(NPAIR):
                    chunks.append((1300.0, pair_norm(p)))
                return chunks

            def out_chunks(ts):
                """Out projection for q/t tile ts (needs norm(ts) done)."""
                tsl = slice(ts * 512, (ts + 1) * 512)
                chunks = []

                def ec_chunk(ec):
                    def fn(ec=ec):
                        ecs = slice(ec * 128, (ec + 1) * 128)
                        pso = ps_sm.tile([128, 512], F32, tag="sm", bufs=2,
                                         name=f"pso{ts}_{ec}")
                        for p in range(NPAIR):
                            nc.tensor.matmul(
                                pso[:], wo_sb[p][:, ecs], qt[p][:, tsl],
                                start=(p == 0), stop=(p == NPAIR - 1),
                            )
                        ot = otpool.tile([128, 512], BF16, tag="ot", bufs=4)
                        nc.vector.tensor_copy(ot[:], pso[:])
                        nc.sync.dma_start(out_d[ecs, tsl], ot[:])
                    return fn

                for ec in range(D // 128):
                    chunks.append((1100.0, ec_chunk(ec)))
                return chunks

            # ---------------- emission schedule ----------------
            xa_t = [xa0, None, None, None]

            def prefetch(ts):
                def fn(ts=ts):
                    xa = xpool.tile([128, ND, 512], BF16, tag="x", bufs=2,
                                    name=f"xa{ts}")
                    xa_t[ts] = xa
                    xa_dma(ts, xa)
                return [(200.0, fn)]

            # QKV(0) sequential (nothing else to overlap yet)
            for _, fn in qkv_chunks(0, xa0):
                fn()
            for _, fn in prefetch(1):
                fn()

            # wo loads: emit after phase-0 DMAs so they don't delay them
            def wo_load():
                for p in range(NPAIR):
                    nc.scalar.dma_start(wo_sb[p][:], woT_d[p])

            # round 0: ATT(0) || QKV(1)
            _merge_emit(att_chunks(0),
                        qkv_chunks(1, xa_t[1]) + prefetch(2) + [(200.0, wo_load)])
            # round 1: ATT(1) || QKV(2) || NORM(0)+OUT(0)
            _merge_emit(att_chunks(1),
                        qkv_chunks(2, xa_t[2]) + prefetch(3),
                        norm_chunks(0) + out_chunks(0))
            # round 2: ATT(2) || QKV(3) || NORM(1)
            _merge_emit(att_chunks(2),
                        qkv_chunks(3, xa_t[3]),
                        norm_chunks(1))
            # round 3: ATT(3) || OUT(1) || NORM(2)+OUT(2)
            _merge_emit(att_chunks(3),
                        out_chunks(1),
                        norm_chunks(2) + out_chunks(2))
            # tail
            for _, fn in norm_chunks(3) + out_chunks(3):
                fn()

    nc.compile()
    return nc


def _get_program():
    global _PROGRAM
    if _PROGRAM is None:
        _PROGRAM = _build_program()
    return _PROGRAM


def _prep_in_maps(in_features, token_positions, W_qkv, W_out):
    in_features = np.asarray(in_features, dtype=np.float32)
    token_positions = np.asarray(token_positions)
    W_qkv = np.asarray(W_qkv, dtype=np.float32)
    W_out = np.asarray(W_out, dtype=np.float32)

    # RoPE pair permutation: [x0 of freq 0..31 | x1 of freq 0..31]
    perm = np.concatenate([np.arange(0, DH, 2), np.arange(1, DH, 2)])

    wqkvT, woT = [], []
    for tp in range(TP):
        rows = []
        for sect in range(2):  # Q, K (permuted)
            for h in range(HLOC):
                g = tp * HLOC + h
                rows.append(W_qkv[sect * D + g * DH + perm])
        for h in range(HLOC):  # V natural
            g = tp * HLOC + h
            rows.append(W_qkv[2 * D + g * DH:2 * D + (g + 1) * DH])
        Wl = np.concatenate(rows, axis=0)  # [1536, 1024]
        wqkvT.append(np.ascontiguousarray(Wl.T).astype(BF))
        woT.append(np.ascontiguousarray(np.stack(
            [np.concatenate([
                W_out[:, (tp * HLOC + 2 * p) * DH:(tp * HLOC + 2 * p + 1) * DH].T,
                W_out[:, (tp * HLOC + 2 * p + 1) * DH:(tp * HLOC + 2 * p + 2) * DH].T,
            ], axis=0) for p in range(NPAIR)])).astype(BF))

    half = DH // 2
    inv_freq = (THETA ** (-2.0 * np.arange(half, dtype=np.float32) / DH)).astype(np.float32)
    ang = token_positions.astype(np.float32)[:, None] * inv_freq[None, :]  # [S, 32]
    cos_t = np.cos(ang).T.astype(np.float32)  # [32, S]
    sin_t = np.sin(ang).T.astype(np.float32)
    cos128 = np.ascontiguousarray(np.tile(cos_t, (4, 1))).astype(BF)
    sin128 = np.ascontiguousarray(
        np.tile(np.concatenate([-sin_t, sin_t], axis=0), (2, 1))).astype(BF)
    # mask[kv, c] = 1 iff kv <= c (scores stored transposed: [kv, q]),
    # duplicated for the two row-packed heads
    m128 = np.triu(np.ones((128, 128), dtype=np.float32))
    mask2 = np.concatenate([m128, m128], axis=1).astype(BF)
    # pair-broadcast one-hot: col (p*128 + c) selects head 2p + c//64
    oh2 = np.zeros((8, 4 * 128), dtype=np.float32)
    for p in range(NPAIR):
        for hh in range(2):
            oh2[2 * p + hh, p * 128 + hh * 64:p * 128 + (hh + 1) * 64] = 1.0

    in_maps = []
    for c in range(NCORES):
        b, tp = c // 2, c % 2
        in_maps.append({
            "xT": np.ascontiguousarray(in_features[b].T).astype(BF),
            "wqkvT": wqkvT[tp],
            "woT": woT[tp],
            "cosT": cos128,
            "sinT": sin128,
            "mask": mask2,
            "oh2": oh2,
        })
    return in_maps


def run(in_features, token_positions, W_qkv, W_out, **spmd_kwargs):
    """Run the kernel; returns (output [B,S,D] f32, BassKernelResults)."""
    in_maps = _prep_in_maps(in_features, token_positions, W_qkv, W_out)
    nc = _get_program()
    res = run_bass_kernel_spmd(nc, in_maps, core_ids=list(range(NCORES)), **spmd_kwargs)
    outs = [res.results[c]["out"].astype(np.float32) for c in range(NCORES)]
    full = np.stack([(outs[2 * b] + outs[2 * b + 1]).T for b in range(B)])
    return full.astype(np.float32), res


def kernel(in_features, token_positions, W_qkv, W_out):
    out, _ = run(in_features, token_positions, W_qkv, W_out)
    return out
